# revision 10
# baseline (speedup 1.0000x reference)
"""Trainium2 Bass kernel for GAT-style single-query attention.

Reference computation (N=16384, D=1024, H=8):
    scores[n,h] = leaky_relu(x0 @ Wi[h] + x[n] @ Wj[h] + b[h], 0.01)
    probs       = softmax(scores, axis=n)  (per head)
    out[d]      = relu(mean_h(sum_n probs[n,h] * x[n,d]))

Strategy: shard rows (N) across 8 cores.  The host pre-casts X to bf16 and
uploads each core's shard in BOTH layouts (natural [2048,1024] and
transposed), each pre-permuted into group-major order so every bulk DMA is
128 contiguous 4KB runs (one per partition) -- descriptor generation on the
HWDGE queue is ~1.2us per instruction otherwise.  This keeps HBM traffic at
8MB/core but removes every big on-device transpose (the f32 version spent
~35us of PE time on X transposes plus ~20us of DVE PSUM->SBUF copies).
The host also folds the tiny per-head constant cvec[h] = x0 @ Wi[h] + b[h]
(16K FLOPs of the 536M total) so no W prep runs on device.

Each core, pipelined over 8 groups of 256 rows:
  - scores^T [8,256] on the PE: wjT[128,8] stationary, X^T bf16 moving,
    accumulated over the 8 d-chunks in fp32 PSUM,
  - u = exp(leaky(s)) as max(exp(s + cvec), exp(0.01 s + 0.01 cvec)) -- the
    cvec bias is applied inside the ACT instruction (per-partition bias AP),
    exp stays on one ACT function table; the DVE max also casts u to bf16
    and accumulates the softmax denominator,
  - u transposed back to natural [128,8] per 128-chunk on the PE (identity
    matmul; tiny), copied to bf16 by the DVE,
  - HO[h,d] += u_chunk^T @ X_chunk on the PE, interleaved into the group
    loop so the PE never sits behind a DMA it doesn't need.
Natural-X DMAs are issued from the scalar-engine HWDGE queue and X^T DMAs
from the sync-engine queue so descriptor generation runs in parallel.
Each core ships its [H, D] partial sums + [H] denominator; the host sums
the 8 partials (33KB each) and finishes relu(mean_h HO_h / Z_h) during the
gather/unshard step.

bf16 inputs give |err| ~3e-3 relative to output scale vs the f32 reference
(scores move by ~0.02 absolute which perturbs probs ~2%, averaged down by
the 16K-row softmax sum).
"""

import sys

sys.path.insert(0, "/opt/trn_rl_repo")

import numpy as np
import ml_dtypes

import concourse.bacc as bacc
import concourse.tile as tile
from concourse import mybir
from concourse import masks
from concourse.bass_utils import run_bass_kernel_spmd

N, D, H = 16384, 1024, 8
NCORES = 8
NSHARD = N // NCORES          # 2048 rows per core
KCH = NSHARD // 128           # 16 n-chunks of 128 rows
DCH = D // 128                # 8 d-chunks of 128
NG = 8                        # pipeline groups
NPG = NSHARD // NG            # 256 rows per group
KPG = KCH // NG               # n-chunks per group (2)
F32 = mybir.dt.float32
BF16 = mybir.dt.bfloat16
AR_W = 1032                   # 1024 head-sums + 1 denom + pad to 32B rows
NPBF16 = ml_dtypes.bfloat16


def _build():
    nc = bacc.Bacc("TRN2", target_bir_lowering=False, debug=False,
                   num_devices=NCORES)
    x_in = nc.dram_tensor("x", [NG, 128, KPG, D], BF16,
                          kind="ExternalInput").ap()
    xt_in = nc.dram_tensor("xt", [NG, 128, DCH, NPG], BF16,
                           kind="ExternalInput").ap()
    wjt_in = nc.dram_tensor("wjt", [128, DCH, H], BF16,
                            kind="ExternalInput").ap()
    cb_in = nc.dram_tensor("cb", [H, 2], F32, kind="ExternalInput").ap()
    out_t = nc.dram_tensor("out", [128, AR_W], F32,
                           kind="ExternalOutput").ap()

    with tile.TileContext(nc) as tc:
        with (
            tc.tile_pool(name="consts", bufs=1) as consts,
            tc.tile_pool(name="small", bufs=1) as small,
            tc.tile_pool(name="xn", bufs=1) as xn_pool,
            tc.tile_pool(name="xt", bufs=1) as xt_pool,
            tc.tile_pool(name="eb", bufs=2) as eb_pool,
            tc.tile_pool(name="pscore", bufs=2, space="PSUM") as pscore_pool,
            tc.tile_pool(name="pu", bufs=2, space="PSUM") as pu_pool,
            tc.tile_pool(name="pho", bufs=1, space="PSUM") as pho_pool,
        ):
            # ---- constants ----
            id128 = consts.tile([128, 128], F32)
            masks.make_identity(nc, id128[:])
            id8 = consts.tile([H, H], BF16)
            nc.vector.tensor_copy(id8[:], id128[:H, :H])

            # ---- small inputs (scalar queue: keep the sync queue free for
            # the first X^T group so compute starts ASAP) ----
            wj_sb = small.tile([128, DCH, H], BF16)
            nc.scalar.dma_start(out=wj_sb[:], in_=wjt_in[:])
            cb_sb = small.tile([H, 2], F32)
            nc.scalar.dma_start(out=cb_sb[:], in_=cb_in[:])

            u_sb = small.tile([H, NSHARD], BF16)
            u_nat = small.tile([128, KCH, H], BF16)
            s_parts = small.tile([H, NG], F32)

            # ---- bulk X DMAs: X^T on the sync queue, X-natural on the
            # scalar queue, both group-major contiguous ----
            xn_t = xn_pool.tile([128, NG, KPG, D], BF16)
            xt_t = xt_pool.tile([128, NG, DCH, NPG], BF16)
            for g in range(NG):
                nc.sync.dma_start(out=xt_t[:, g], in_=xt_in[g])
                nc.scalar.dma_start(out=xn_t[:, g], in_=x_in[g])

            # ---- main pipeline ----
            # HO is col-tiled 4-way: chunk j accumulates into PSUM rows
            # 32*(j%4)+h, so 4 chunks' matmuls run concurrently in distinct
            # 32-column strips of the PE array.  Rows the matmuls never
            # touch are zeroed up front (also keeps the final full-tile
            # copy reading initialized PSUM); the host sums the 4 strips.
            ho0 = pho_pool.tile([128, 512], F32, tag="ho0")
            ho1 = pho_pool.tile([128, 512], F32, tag="ho1")
            nc.vector.memset(ho0[:], 0.0)
            nc.vector.memset(ho1[:], 0.0)
            for g in range(NG):
                # scores^T [8, NPG] += wjT_c^T @ X^T_c
                ps = pscore_pool.tile([H, NPG], F32, tag="ps")
                for c in range(DCH):
                    nc.tensor.matmul(
                        ps[:], wj_sb[:, c, :], xt_t[:, g, c, :],
                        start=(c == 0), stop=(c == DCH - 1))

                # u = exp(leaky(s + cvec)) = max(exp(s+cvec), exp(.01(s+cvec)))
                e1 = eb_pool.tile([H, NPG], F32, tag="e1")
                nc.scalar.activation(
                    e1[:], ps[:], mybir.ActivationFunctionType.Exp,
                    bias=cb_sb[:, 0:1], scale=1.0)
                e2 = eb_pool.tile([H, NPG], F32, tag="e2")
                nc.scalar.activation(
                    e2[:], ps[:], mybir.ActivationFunctionType.Exp,
                    bias=cb_sb[:, 1:2], scale=0.01)
                nc.vector.scalar_tensor_tensor(
                    u_sb[:, g * NPG:(g + 1) * NPG], e1[:], 1.0, e2[:],
                    mybir.AluOpType.mult, mybir.AluOpType.max,
                    accum_out=s_parts[:, g:g + 1])

                # u back to natural layout (PE identity transpose), then HO
                for jj in range(KPG):
                    j = g * KPG + jj
                    t = j % 4
                    pu = pu_pool.tile([128, H], BF16, tag="pu")
                    nc.tensor.transpose(
                        pu[:], u_sb[:, j * 128:(j + 1) * 128], id8[:])
                    nc.vector.tensor_copy(u_nat[:, j, :], pu[:])
                    nc.tensor.matmul(
                        ho0[32 * t:32 * t + H, :], u_nat[:, j, :],
                        xn_t[:, g, jj, 0:512], tile_position=(0, 32 * t),
                        start=(j == t), stop=(j == 12 + t))
                    nc.tensor.matmul(
                        ho1[32 * t:32 * t + H, :], u_nat[:, j, :],
                        xn_t[:, g, jj, 512:1024], tile_position=(0, 32 * t),
                        start=(j == t), stop=(j == 12 + t))

            # ---- payload: [128, 1024 col-tiled HO | 1 Z | pad]; the two
            # PSUM->SBUF copies split across DVE and ACT so the tail is
            # one copy long, not two ----
            ar_sb = small.tile([128, AR_W], F32)
            nc.vector.memset(ar_sb[:, 1024:], 0.0)
            nc.vector.tensor_copy(ar_sb[:, 0:512], ho0[:])
            nc.scalar.activation(ar_sb[:, 512:1024], ho1[:],
                                 mybir.ActivationFunctionType.Copy)
            nc.vector.tensor_reduce(ar_sb[0:H, 1024:1025], s_parts[:],
                                    axis=mybir.AxisListType.X,
                                    op=mybir.AluOpType.add)
            nc.sync.dma_start(out=out_t[:], in_=ar_sb[:])

    nc.compile()
    return nc


_CACHE = {}


def _get_program():
    if "nc" not in _CACHE:
        _CACHE["nc"] = _build()
    return _CACHE["nc"]


def _in_maps(final_result, W, b):
    X = np.ascontiguousarray(final_result, dtype=np.float32)
    W = np.ascontiguousarray(W, dtype=np.float32)
    b = np.asarray(b, dtype=np.float32).reshape(H)
    Xbf = X.astype(NPBF16)
    wjt = np.ascontiguousarray(
        W[:, D:].astype(NPBF16).T.reshape(DCH, 128, H).transpose(1, 0, 2))
    cvec = X[0] @ W[:, :D].T + b
    cb = np.stack([cvec, 0.01 * cvec], axis=1).astype(np.float32)
    maps = []
    for c in range(NCORES):
        xn = Xbf[c * NSHARD:(c + 1) * NSHARD]          # [2048, 1024]
        # x[g, p, k, d] = xn[(g*KPG + k)*128 + p, d]
        x4 = np.ascontiguousarray(
            xn.reshape(NG, KPG, 128, D).transpose(0, 2, 1, 3))
        # xt[g, p, c, nl] = xn[g*NPG + nl, c*128 + p]
        xt4 = np.ascontiguousarray(
            xn.reshape(NG, NPG, DCH, 128).transpose(0, 3, 2, 1))
        maps.append({"x": x4, "xt": xt4, "wjt": wjt, "cb": cb})
    return maps


def _finalize(ar):
    # rows 32t+h hold col-tile t's partial of head h
    ho = sum(ar[32 * t:32 * t + H, 0:D] for t in range(4))
    z = ar[0:H, D:D + 1]
    r = (ho / (H * z)).sum(axis=0, dtype=np.float32)
    return np.maximum(r, np.float32(0)).astype(np.float32)


def kernel(final_result, W, b):
    nc = _get_program()
    res = run_bass_kernel_spmd(nc, _in_maps(final_result, W, b),
                               list(range(NCORES)))
    parts = [np.asarray(res.results[c]["out"], dtype=np.float32)
             for c in range(NCORES)]
    return _finalize(np.sum(parts, axis=0, dtype=np.float32))


if __name__ == "__main__":
    rng = np.random.default_rng(0)
    x = rng.standard_normal((N, D), dtype=np.float32)
    W = (rng.standard_normal((H, 2 * D)) * 0.05).astype(np.float32)
    b = (rng.standard_normal(H) * 0.05).astype(np.float32)
    out = kernel(final_result=x, W=W, b=b)
    print("kernel out:", out.shape, out[:8])
